# revision 13
# baseline (speedup 1.0000x reference)
"""Complex Conv1D (VALID, stride 1) on Trainium2 — Bass/Tile, 8-core data-parallel.

Problem (hardcoded shapes):
  x_real/x_imag: [32, 4096, 64] f32, kernel_real/imag: [9, 64, 64] f32,
  bias_real/imag: [64] f32  ->  out [32, 4088, 64, 2] f32
  out_real = conv(xr, wr) - conv(xi, wi) + br
  out_imag = conv(xr, wi) + conv(xi, wr) + bi

Mapping: complex multiply as its 2x2 real block-matrix form so each tap is ONE
full 128-contract matmul:
  X_b [128, L]   rows 0:64 = xr[b].T (channels on partitions), 64:128 = xi[b].T
  W[k] [128,128] = [[wr[k], wi[k]], [-wi[k], wr[k]]]
  psum[128, T] += W[k].T @ X_b[:, l0+k : l0+k+T]   for k = 0..8
  psum rows 0:64 = real output (filters), rows 64:128 = imag output.
Batch is sharded 4-per-core across 8 cores; weights replicated. The kernel
emits the output transposed as [b, 128, L_out]; the host restores
[B, L_out, F, 2].

Schedule: one whole-batch x DMA in, 8 psum tiles of 512 columns each
(9 accumulating matmuls per tile), activation-engine evacuation with bias into
a whole-batch SBUF output tile, one whole-batch DMA out. DMA instruction count
is the key scalar cost: each dma_start occupies the issuing sequencer for
~0.6-1.2us, so bulk transfers keep the SP queue far off the critical path.
"""

import numpy as np

import concourse.bacc as bacc
import concourse.bass as bass
import concourse.mybir as mybir
from concourse.tile import TileContext
from concourse.bass_utils import run_bass_kernel_spmd

B, L, CIN, KT, F = 32, 4096, 64, 9, 64
LOUT = L - KT + 1  # 4088
NCORES = 8
BPC = B // NCORES  # batches per core
TL = 512  # output-tile width (one PSUM bank of fp32)
NLT = (LOUT + TL - 1) // TL  # 8

# Matmul operand dtype: float32r streams fp32 operands through the PE in a
# single reduced-precision pass (1 cycle/row for free dim >= 256); bfloat16
# matches that rate but halves DMA traffic at ~bf16 accuracy (rel err ~3e-3).
MM_DT_NAME = "bfloat16"
OUT_DT_NAME = "bfloat16"


def _build_nc(
    mm_dt,
    w_dt=None,
    out_dt=None,
    xbufs=2,
    obufs=2,
    psbufs=6,
    warmup=28,
    first_xc=TL + KT - 1,
    out_split=6 * TL - 6,
    last_split=7 * TL - 7,
    loop_repeat=None,
):
    nc = bacc.Bacc("TRN2", target_bir_lowering=False, debug=False, num_devices=NCORES)
    if w_dt is None:
        w_dt = mm_dt
    f32 = mybir.dt.float32
    if out_dt is None:
        out_dt = f32
    ident = mybir.ActivationFunctionType.Identity

    x_d = nc.dram_tensor("x", [BPC, 128, L], mm_dt, kind="ExternalInput")
    w_d = nc.dram_tensor("w", [128, KT * 128], w_dt, kind="ExternalInput")
    bias_d = nc.dram_tensor("bias", [128, 1], f32, kind="ExternalInput")
    out_d = nc.dram_tensor("out", [BPC, 128, LOUT], out_dt, kind="ExternalOutput")

    with TileContext(nc) as tc:
        with (
            tc.tile_pool(name="wpool", bufs=1) as wpool,
            tc.tile_pool(name="xpool", bufs=xbufs) as xpool,
            tc.tile_pool(name="opool", bufs=obufs) as opool,
            tc.tile_pool(name="pspool", bufs=psbufs, space="PSUM") as pspool,
        ):
            # First x chunk of batch 0 is small so the first real matmul group
            # can start ~2-3us in; warmup matmuls on a throwaway tile keep the
            # PE busy (pstate ramp) until then. x chunk layout per batch:
            #   batch 0:  [0, first_xc), [first_xc, mid), [mid, L)
            #   batch 1+: [0, L/2), [L/2, L)
            wt = wpool.tile([128, KT * 128], w_dt)
            bias_t = wpool.tile([128, 1], f32)
            if loop_repeat is None:
                xt0 = xpool.tile([128, L], mm_dt, tag="xt")
                nc.sync.dma_start(xt0[:, :first_xc], x_d[0, :, :first_xc])
            # Weights ride the Activation queue so they land in parallel with
            # the first x chunk on SP.
            nc.scalar.dma_start(wt[:], w_d[:])
            nc.scalar.dma_start(bias_t[:], bias_d[:])

            if warmup:
                dummy = wpool.tile([128, 128], mm_dt)
                nc.vector.memset(dummy[:], 0.0)
                wps = pspool.tile([128, 128], f32, tag="wps", bufs=1)
                for _ in range(warmup):
                    nc.tensor.matmul(
                        wps[:], dummy[:], dummy[:],
                        start=True, stop=True, skip_group_check=True,
                    )

            if loop_repeat is None:
                mid = (first_xc + L) // 2
                nc.sync.dma_start(xt0[:, first_xc:mid], x_d[0, :, first_xc:mid])
                nc.sync.dma_start(xt0[:, mid:], x_d[0, :, mid:])

            import contextlib

            loop_cm = (
                tc.For_i(0, loop_repeat, 1)
                if loop_repeat is not None
                else contextlib.nullcontext()
            )
            with loop_cm:
                for b in range(BPC):
                    if b == 0 and loop_repeat is None:
                        xt = xt0
                    else:
                        xt = xpool.tile([128, L], mm_dt, tag="xt")
                        nc.sync.dma_start(xt[:, : L // 2], x_d[b, :, : L // 2])
                        nc.sync.dma_start(xt[:, L // 2 :], x_d[b, :, L // 2 :])
                    ot = opool.tile([128, LOUT], out_dt, tag="ot")
                    for j in range(NLT):
                        l0 = j * TL
                        t = min(TL, LOUT - l0)
                        ps = pspool.tile([128, TL], f32, tag="ps")
                        for k in range(KT):
                            nc.tensor.matmul(
                                ps[:, :t],
                                wt[:, k * 128 : (k + 1) * 128],
                                xt[:, l0 + k : l0 + k + t],
                                start=(k == 0),
                                stop=(k == KT - 1),
                            )
                        nc.scalar.activation(
                            ot[:, l0 : l0 + t], ps[:, :t], ident, bias=bias_t[:]
                        )
                    # Output stores ride the Activation HWDGE queue; the final
                    # chunk is small to shorten the drain tail (smaller still
                    # for the last batch, which IS the tail).
                    if b == BPC - 1 and loop_repeat is None:
                        # Last batch: big store on SP (idle by now) so the
                        # final evacs aren't queued behind it on Act.
                        nc.sync.dma_start(
                            out_d[b, :, :out_split], ot[:, :out_split]
                        )
                    else:
                        nc.scalar.dma_start(
                            out_d[b, :, :out_split], ot[:, :out_split]
                        )
                    if b == BPC - 1 and loop_repeat is None:
                        nc.scalar.dma_start(
                            out_d[b, :, out_split:last_split],
                            ot[:, out_split:last_split],
                        )
                        nc.scalar.dma_start(
                            out_d[b, :, last_split:], ot[:, last_split:]
                        )
                    else:
                        nc.scalar.dma_start(
                            out_d[b, :, out_split:], ot[:, out_split:]
                        )

    nc.compile()
    return nc


def _pack(x_real, x_imag, kernel_real, kernel_imag, bias_real, bias_imag, np_dt,
          w_np_dt=None):
    if w_np_dt is None:
        w_np_dt = np_dt
    X = np.empty((B, 128, L), np_dt)
    X[:, :CIN] = x_real.transpose(0, 2, 1)
    X[:, CIN:] = x_imag.transpose(0, 2, 1)
    Wk = np.empty((KT, 128, 128), np.float32)
    Wk[:, :CIN, :F] = kernel_real
    Wk[:, :CIN, F:] = kernel_imag
    Wk[:, CIN:, :F] = -kernel_imag
    Wk[:, CIN:, F:] = kernel_real
    W2 = Wk.transpose(1, 0, 2).reshape(128, KT * 128).astype(w_np_dt)
    bias2 = (
        np.concatenate([bias_real, bias_imag]).reshape(128, 1).astype(np.float32)
    )
    return X, np.ascontiguousarray(W2), bias2


def _parse_dt(name):
    name = name or MM_DT_NAME
    parts = name.split(",")
    xn = parts[0]
    wn = parts[1] if len(parts) > 1 else xn
    on = parts[2] if len(parts) > 2 else OUT_DT_NAME
    return getattr(mybir.dt, xn), getattr(mybir.dt, wn), getattr(mybir.dt, on)


def _prepare(inputs, mm_dt_name=None, build_kw=None):
    mm_dt, w_dt, out_dt = _parse_dt(mm_dt_name)
    np_dt = mybir.dt.np(mm_dt)
    w_np_dt = mybir.dt.np(w_dt)
    args = {
        k: np.asarray(inputs[k], np.float32)
        for k in (
            "x_real", "x_imag", "kernel_real", "kernel_imag", "bias_real", "bias_imag",
        )
    }
    X, W2, bias2 = _pack(np_dt=np_dt, w_np_dt=w_np_dt, **args)

    nc = _build_nc(mm_dt, w_dt=w_dt, out_dt=out_dt, **(build_kw or {}))
    in_maps = [
        {
            "x": np.ascontiguousarray(X[i * BPC : (i + 1) * BPC]),
            "w": W2,
            "bias": bias2,
        }
        for i in range(NCORES)
    ]
    return nc, in_maps


def _gather(results):
    O = np.concatenate([r["out"] for r in results], axis=0)  # [32, 128, 4088]
    O = O.astype(np.float32).reshape(B, 2, F, LOUT).transpose(0, 3, 2, 1)
    return np.ascontiguousarray(O, dtype=np.float32)


def _run(inputs, trace=False, mm_dt_name=None):
    nc, in_maps = _prepare(inputs, mm_dt_name)
    res = run_bass_kernel_spmd(nc, in_maps, core_ids=list(range(NCORES)), trace=trace)
    return _gather(res.results), res


def kernel(**inputs) -> np.ndarray:
    out, _ = _run(inputs, trace=False)
    return out


# revision 35
# speedup vs baseline: 1.2475x; 1.2475x over previous
"""Complex Conv1D (VALID, stride 1) on Trainium2 — Bass/Tile, 8-core data-parallel.

Problem (hardcoded shapes):
  x_real/x_imag: [32, 4096, 64] f32, kernel_real/imag: [9, 64, 64] f32,
  bias_real/imag: [64] f32  ->  out [32, 4088, 64, 2] f32
  out_real = conv(xr, wr) - conv(xi, wi) + br
  out_imag = conv(xr, wi) + conv(xi, wr) + bi

Mapping: complex multiply as its 2x2 real block-matrix form so each tap is ONE
full 128-contract matmul:
  X_b [128, L]   rows 0:64 = xr[b].T (channels on partitions), 64:128 = xi[b].T
  W[k] [128,128] = [[wr[k], wi[k]], [-wi[k], wr[k]]]
  psum[128, T] += W[k].T @ X_b[:, l0+k : l0+k+T]   for k = 0..8
  psum rows 0:64 = real output (filters), rows 64:128 = imag output.
Batch is sharded 4-per-core across 8 cores; weights replicated. The kernel
emits the output transposed; the host restores [B, L_out, F, 2].

Schedule notes (measured on the axon-tunneled TRN2 cores):
- Per-DMA fixed cost is ~3us and effective HBM bandwidth ~200 GB/s, far below
  the 400 GB/s spec, so the kernel minimizes DMA instructions: batches are
  packed in PAIRS on the host (x: [2, 128, 2*4096] per core) giving 2 x-loads
  + 2 stores + weights + bias = 6 DMAs per core.
- bf16 operands/outputs halve DMA bytes (rel err ~3e-3, tolerance 2e-2).
- PE floor is 288 matmuls x 512 rows @ 2.4 GHz ~= 61us; evacuation runs on
  the DVE engine so the Activation queue never blocks PSUM drains.
- Warmup matmuls on a dummy tile hold the PE pstate ramp until the first x
  chunk lands (single-shot build loads a small first chunk for fast start).
"""

import numpy as np

import concourse.bacc as bacc
import concourse.bass as bass
import concourse.mybir as mybir
from concourse.tile import TileContext
from concourse.bass_utils import run_bass_kernel_spmd

B, L, CIN, KT, F = 32, 4096, 64, 9, 64
LOUT = L - KT + 1  # 4088
NCORES = 8
BPC = B // NCORES  # batches per core
PAIR = 2
NP = BPC // PAIR  # batch-pairs per core
TL = 512  # output-tile width (one PSUM bank of fp32)
NLT = (LOUT + TL - 1) // TL  # 8

MM_DT_NAME = "bfloat16"
OUT_DT_NAME = "bfloat16"


def _build_nc(
    mm_dt,
    w_dt=None,
    out_dt=None,
    xbufs=2,
    obufs=2,
    psbufs=6,
    warmup=28,
    first_xc=TL + KT - 1,
    last_split=7 * TL - 7,
    evac="dve",  # act | dve | alt
    mode="full",  # full | pe_only (no DMA) | no_store | dma_only (no compute)
    store_q="act",  # act | pool | sp
    nxd=1,  # x-load DMAs per batch pair
    loop_repeat=None,
):
    nc = bacc.Bacc("TRN2", target_bir_lowering=False, debug=False, num_devices=NCORES)
    if w_dt is None:
        w_dt = mm_dt
    f32 = mybir.dt.float32
    if out_dt is None:
        out_dt = f32
    ident = mybir.ActivationFunctionType.Identity

    x_d = nc.dram_tensor("x", [NP, 128, PAIR * L], mm_dt, kind="ExternalInput")
    w_d = nc.dram_tensor("w", [128, KT * 128], w_dt, kind="ExternalInput")
    bias_d = nc.dram_tensor("bias", [128, 1], f32, kind="ExternalInput")
    out_d = nc.dram_tensor(
        "out", [NP, 128, PAIR * LOUT], out_dt, kind="ExternalOutput"
    )

    with TileContext(nc) as tc:
        with (
            tc.tile_pool(name="wpool", bufs=1) as wpool,
            tc.tile_pool(name="xpool", bufs=xbufs) as xpool,
            tc.tile_pool(name="opool", bufs=obufs) as opool,
            tc.tile_pool(name="pspool", bufs=psbufs, space="PSUM") as pspool,
        ):
            wt = wpool.tile([128, KT * 128], w_dt)
            bias_t = wpool.tile([128, 1], f32)
            if mode != "pe_only" and loop_repeat is None:
                # Small first chunk so real matmuls can start ~3us in.
                xt0 = xpool.tile([128, PAIR * L], mm_dt, tag="xt")
                nc.sync.dma_start(xt0[:, :first_xc], x_d[0, :, :first_xc])
            nc.scalar.dma_start(wt[:], w_d[:])
            nc.scalar.dma_start(bias_t[:], bias_d[:])

            if warmup:
                # bf16 regardless of mm_dt: memset can't target float32r.
                dummy = wpool.tile([128, 128], mybir.dt.bfloat16)
                nc.vector.memset(dummy[:], 0.0)
                wps = pspool.tile([128, TL], f32, tag="ps")
                for _ in range(warmup):
                    nc.tensor.matmul(
                        wps[:, :128], dummy[:], dummy[:],
                        start=True, stop=True, skip_group_check=True,
                    )

            xts = None
            if mode != "pe_only" and loop_repeat is None:
                nc.sync.dma_start(xt0[:, first_xc:L], x_d[0, :, first_xc:L])
                nc.sync.dma_start(xt0[:, L:], x_d[0, :, L:])
                # Preload the remaining pairs too: all x traffic lands during
                # the first pair's compute, the rest runs DMA-free.
                xts = [xt0]
                for pi in range(1, NP):
                    xt_n = xpool.tile([128, PAIR * L], mm_dt, tag="xt")
                    nc.sync.dma_start(xt_n[:], x_d[pi])
                    xts.append(xt_n)

            if mode == "pe_only":
                xt_fix = xpool.tile([128, PAIR * L], mm_dt, tag="xt")
                nc.sync.dma_start(xt_fix[:], x_d[0])
            if mode == "dma_only":
                ot_fix = opool.tile([128, PAIR * LOUT], out_dt, tag="ot")
                nc.vector.memset(ot_fix[:], 0.0)

            import contextlib

            loop_cm = (
                tc.For_i(0, loop_repeat, 1)
                if loop_repeat is not None
                else contextlib.nullcontext()
            )
            st_eng = {"act": nc.scalar, "pool": nc.gpsimd, "sp": nc.sync}[store_q]
            with loop_cm:
                for pi in range(NP):
                    if mode == "pe_only":
                        xt = xt_fix
                    elif loop_repeat is None:
                        xt = xts[pi]
                    else:
                        xt = xpool.tile([128, PAIR * L], mm_dt, tag="xt")
                        if nxd == 1:
                            nc.sync.dma_start(xt[:], x_d[pi])
                        else:
                            xc = (PAIR * L + nxd - 1) // nxd
                            for i in range(nxd):
                                lo, hi = i * xc, min(PAIR * L, (i + 1) * xc)
                                nc.sync.dma_start(xt[:, lo:hi], x_d[pi, :, lo:hi])
                    ot = (
                        ot_fix
                        if mode == "dma_only"
                        else opool.tile([128, PAIR * LOUT], out_dt, tag="ot")
                    )
                    if mode != "dma_only":
                        for sb in range(PAIR):
                            xb, ob = sb * L, sb * LOUT
                            for j in range(NLT):
                                l0 = j * TL
                                t = min(TL, LOUT - l0)
                                ps = pspool.tile([128, TL], f32, tag="ps")
                                for k in range(KT):
                                    nc.tensor.matmul(
                                        ps[:, :t],
                                        wt[:, k * 128 : (k + 1) * 128],
                                        xt[:, xb + l0 + k : xb + l0 + k + t],
                                        start=(k == 0),
                                        stop=(k == KT - 1),
                                    )
                                if evac == "dve" or (evac == "alt" and j % 2):
                                    nc.vector.tensor_scalar_add(
                                        ot[:, ob + l0 : ob + l0 + t],
                                        ps[:, :t],
                                        bias_t[:],
                                    )
                                else:
                                    nc.scalar.activation(
                                        ot[:, ob + l0 : ob + l0 + t],
                                        ps[:, :t],
                                        ident,
                                        bias=bias_t[:],
                                    )
                    if mode in ("pe_only", "no_store"):
                        continue
                    if pi == NP - 1 and loop_repeat is None:
                        # Drain tail: bulk of the last pair early via SP, a
                        # short final chunk behind the last evac.
                        cut = LOUT + last_split
                        nc.sync.dma_start(out_d[pi, :, :cut], ot[:, :cut])
                        st_eng.dma_start(out_d[pi, :, cut:], ot[:, cut:])
                    else:
                        st_eng.dma_start(out_d[pi], ot[:])

    nc.compile()
    return nc


def _pack(x_real, x_imag, kernel_real, kernel_imag, bias_real, bias_imag, np_dt,
          w_np_dt=None):
    if w_np_dt is None:
        w_np_dt = np_dt
    X = np.empty((B, 128, L), np.float32)
    X[:, :CIN] = x_real.transpose(0, 2, 1)
    X[:, CIN:] = x_imag.transpose(0, 2, 1)
    # Pack batch pairs side by side: XP[p, :, sb*L:(sb+1)*L] = X[2p+sb]
    XP = X.reshape(B // PAIR, PAIR, 128, L).transpose(0, 2, 1, 3).reshape(
        B // PAIR, 128, PAIR * L
    )
    Wk = np.empty((KT, 128, 128), np.float32)
    Wk[:, :CIN, :F] = kernel_real
    Wk[:, :CIN, F:] = kernel_imag
    Wk[:, CIN:, :F] = -kernel_imag
    Wk[:, CIN:, F:] = kernel_real
    W2 = Wk.transpose(1, 0, 2).reshape(128, KT * 128).astype(w_np_dt)
    bias2 = (
        np.concatenate([bias_real, bias_imag]).reshape(128, 1).astype(np.float32)
    )
    return (
        np.ascontiguousarray(XP.astype(np_dt)),
        np.ascontiguousarray(W2),
        bias2,
    )


def _parse_dt(name):
    name = name or MM_DT_NAME
    parts = name.split(",")
    xn = parts[0]
    wn = parts[1] if len(parts) > 1 else xn
    on = parts[2] if len(parts) > 2 else OUT_DT_NAME
    return getattr(mybir.dt, xn), getattr(mybir.dt, wn), getattr(mybir.dt, on)


def _prepare(inputs, mm_dt_name=None, build_kw=None):
    mm_dt, w_dt, out_dt = _parse_dt(mm_dt_name)
    np_dt = mybir.dt.np(mm_dt)
    w_np_dt = mybir.dt.np(w_dt)
    args = {
        k: np.asarray(inputs[k], np.float32)
        for k in (
            "x_real", "x_imag", "kernel_real", "kernel_imag", "bias_real", "bias_imag",
        )
    }
    XP, W2, bias2 = _pack(np_dt=np_dt, w_np_dt=w_np_dt, **args)

    nc = _build_nc(mm_dt, w_dt=w_dt, out_dt=out_dt, **(build_kw or {}))
    in_maps = [
        {
            "x": np.ascontiguousarray(XP[i * NP : (i + 1) * NP]),
            "w": W2,
            "bias": bias2,
        }
        for i in range(NCORES)
    ]
    return nc, in_maps


def _gather(results):
    O = np.concatenate([r["out"] for r in results], axis=0)  # [B/2, 128, 2*LOUT]
    O = O.astype(np.float32).reshape(B // PAIR, 2, F, PAIR, LOUT)
    O = O.transpose(0, 3, 4, 2, 1).reshape(B, LOUT, F, 2)
    return np.ascontiguousarray(O, dtype=np.float32)


def _run(inputs, trace=False, mm_dt_name=None):
    nc, in_maps = _prepare(inputs, mm_dt_name)
    res = run_bass_kernel_spmd(nc, in_maps, core_ids=list(range(NCORES)), trace=trace)
    return _gather(res.results), res


def kernel(**inputs) -> np.ndarray:
    out, _ = _run(inputs, trace=False)
    return out
